# revision 1
# baseline (speedup 1.0000x reference)
"""GraphTransformer 2-layer (TransformerConv x2) on 8 Trainium2 NeuronCores.

Sharding: destination-node partitioning with degree-sorted padded tiles.
  - Pad N=50000 -> N'=50176 (392 tiles of 128 nodes). Sort nodes by
    in-degree, bin-pack the tiles onto 8 cores (49 each, balancing slots).
  - Each core receives x^T in a per-core *rotated* node order (its own
    nodes first), computes the full layer-1 K|V table [N',512] on-device
    (replicated compute beats moving 100 MB), plus Q|S for its own nodes.
  - Attention per dst-tile: for neighbor-rank d an indirect DMA gathers
    the d-th neighbor's kv row for all 128 nodes (one row per partition).
    Padding slots point at row 0 and add -1e30 to the logit, so softmax
    kills them. Segment softmax is then plain free-dim reductions.
  - Layer-2 K|V [N',20] is computed from the local h chunk and AllGathered
    (4 MB on the wire instead of 51 MB of h).
All indices/degrees/tile shapes are baked in at build time from the actual
inputs. kernel() builds + runs the single-launch SPMD program and
unpermutes the output on the host.
"""

import numpy as np

N_CORES = 8
N = 50000
IN_DIM = 128
D1 = 256            # heads*hid layer1
H1, C1 = 8, 32
D2 = 10             # layer2 out channels (1 head)
P = 128
NEG = -1.0e30


def _plan(edge_index):
    src = np.asarray(edge_index[0], dtype=np.int64)
    dst = np.asarray(edge_index[1], dtype=np.int64)
    deg = np.bincount(dst, minlength=N)
    NP_ = ((N + N_CORES * P - 1) // (N_CORES * P)) * (N_CORES * P)  # 50176
    n_tiles = NP_ // P                                              # 392
    per_core = n_tiles // N_CORES                                   # 49

    degp = np.concatenate([deg, np.zeros(NP_ - N, np.int64)])
    order0 = np.argsort(degp, kind="stable")        # old(padded) ids, deg asc
    tile_of = order0.reshape(n_tiles, P)            # prelim tile -> old ids
    tile_D = degp[tile_of].max(axis=1)

    # bin-pack tiles onto cores: largest-first greedy with capacity
    t_order = np.argsort(-tile_D, kind="stable")
    loads = np.zeros(N_CORES, np.int64)
    counts = np.zeros(N_CORES, np.int64)
    assign = [[] for _ in range(N_CORES)]
    for t in t_order:
        open_cores = [c for c in range(N_CORES) if counts[c] < per_core]
        c = min(open_cores, key=lambda cc: (loads[cc], cc))
        assign[c].append(int(t))
        loads[c] += int(tile_D[t])
        counts[c] += 1
    for c in range(N_CORES):
        assign[c].sort(key=lambda t: int(tile_D[t]))

    final_tiles = [t for c in range(N_CORES) for t in assign[c]]
    perm = tile_of[final_tiles].reshape(-1)         # new id -> old(padded) id
    inv = np.empty(NP_, np.int64)
    inv[perm] = np.arange(NP_)

    Ds = degp[perm].reshape(n_tiles, P).max(axis=1).astype(np.int64)

    # per-(new)tile neighbor tables in NEW ids; pad idx=0, bias=NEG
    dst_new = inv[dst]
    src_new = inv[src]
    eo = np.argsort(dst_new, kind="stable")
    dst_s = dst_new[eo]
    src_s = src_new[eo]
    row_start = np.searchsorted(dst_s, np.arange(NP_))
    row_end = np.searchsorted(dst_s, np.arange(NP_) + 1)

    idx_tiles, bias_tiles = [], []
    for t in range(n_tiles):
        D = int(Ds[t])
        it = np.zeros((P, D), np.int64)
        bt = np.full((P, D), NEG, np.float32)
        for p in range(P):
            s, e = row_start[t * P + p], row_end[t * P + p]
            k = e - s
            it[p, :k] = src_s[s:e]
            bt[p, :k] = 0.0
        idx_tiles.append(it)
        bias_tiles.append(bt)

    return dict(NP=NP_, n_tiles=n_tiles, per_core=per_core, perm=perm,
                inv=inv, Ds=[int(d) for d in Ds], idx_tiles=idx_tiles,
                bias_tiles=bias_tiles)


def _build_program(NP_, per_core, Ds_pos, biases_zero, sim1=False):
    import concourse.bass as bass
    import concourse.mybir as mybir
    from concourse import bacc
    from concourse.tile import TileContext
    from concourse.masks import make_identity

    f32 = mybir.dt.float32
    i32 = mybir.dt.int32
    NOWN = per_core * P
    slots = sum(P * d for d in Ds_pos)
    Dmax = max(Ds_pos)
    slot_off = [0]
    for j in range(per_core):
        slot_off.append(slot_off[-1] + P * Ds_pos[j])

    nc = bacc.Bacc("TRN2", target_bir_lowering=False, debug=False,
                   num_devices=1 if sim1 else N_CORES)

    xT = nc.dram_tensor("xT", [IN_DIM, NP_], f32, kind="ExternalInput")
    w_kv1 = nc.dram_tensor("w_kv1", [IN_DIM, 2 * D1], f32, kind="ExternalInput")
    w_qs1 = nc.dram_tensor("w_qs1", [IN_DIM, 2 * D1], f32, kind="ExternalInput")
    w_kv2 = nc.dram_tensor("w_kv2", [D1, 2 * D2], f32, kind="ExternalInput")
    w_qs2 = nc.dram_tensor("w_qs2", [D1, 2 * D2], f32, kind="ExternalInput")
    b_kv1 = nc.dram_tensor("b_kv1", [1, 2 * D1], f32, kind="ExternalInput")
    b_qs1 = nc.dram_tensor("b_qs1", [1, 2 * D1], f32, kind="ExternalInput")
    b_kv2 = nc.dram_tensor("b_kv2", [1, 2 * D2], f32, kind="ExternalInput")
    b_qs2 = nc.dram_tensor("b_qs2", [1, 2 * D2], f32, kind="ExternalInput")
    idx1_f = nc.dram_tensor("idx1_f", [slots], i32, kind="ExternalInput")
    idx2_f = nc.dram_tensor("idx2_f", [slots], i32, kind="ExternalInput")
    bias_f = nc.dram_tensor("bias_f", [slots], f32, kind="ExternalInput")
    out_d = nc.dram_tensor("out", [NOWN, D2], f32, kind="ExternalOutput")

    kv1_t = nc.dram_tensor("kv1_t", [NP_, 2 * D1], f32)
    qs1_t = nc.dram_tensor("qs1_t", [NOWN, 2 * D1], f32)
    h_t = nc.dram_tensor("h_t", [NOWN, D1], f32)
    kv2_own = nc.dram_tensor("kv2_own", [NOWN, 2 * D2], f32)
    qs2_t = nc.dram_tensor("qs2_t", [NOWN, 2 * D2], f32)
    kv2_full = nc.dram_tensor("kv2_full", [NP_, 2 * D2], f32, addr_space="Shared")

    X = mybir.AxisListType.X
    MUL = mybir.AluOpType.mult
    ADD = mybir.AluOpType.add
    SUB = mybir.AluOpType.subtract
    EXP = mybir.ActivationFunctionType.Exp
    RELU = mybir.ActivationFunctionType.Relu

    with TileContext(nc) as tc:
        with tc.tile_pool(name="wpool", bufs=1) as wpool:
            w_kv1_s = wpool.tile([IN_DIM, 2 * D1], f32)
            nc.sync.dma_start(out=w_kv1_s[:], in_=w_kv1[:, :])
            w_qs1_s = wpool.tile([IN_DIM, 2 * D1], f32)
            nc.sync.dma_start(out=w_qs1_s[:], in_=w_qs1[:, :])
            w_kv2_s = wpool.tile([P, 2 * (2 * D2)], f32)
            nc.sync.dma_start(out=w_kv2_s[:, 0:2 * D2], in_=w_kv2[0:P, :])
            nc.sync.dma_start(out=w_kv2_s[:, 2 * D2:4 * D2], in_=w_kv2[P:2 * P, :])
            w_qs2_s = wpool.tile([P, 2 * (2 * D2)], f32)
            nc.sync.dma_start(out=w_qs2_s[:, 0:2 * D2], in_=w_qs2[0:P, :])
            nc.sync.dma_start(out=w_qs2_s[:, 2 * D2:4 * D2], in_=w_qs2[P:2 * P, :])
            if not biases_zero:
                ones1 = wpool.tile([1, P], f32)
                nc.vector.memset(ones1[:], 1.0)
                b_kv1_s = wpool.tile([1, 2 * D1], f32)
                nc.sync.dma_start(out=b_kv1_s[:], in_=b_kv1[:, :])
                b_qs1_s = wpool.tile([1, 2 * D1], f32)
                nc.sync.dma_start(out=b_qs1_s[:], in_=b_qs1[:, :])
                b_kv2_s = wpool.tile([1, 2 * D2], f32)
                nc.sync.dma_start(out=b_kv2_s[:], in_=b_kv2[:, :])
                b_qs2_s = wpool.tile([1, 2 * D2], f32)
                nc.sync.dma_start(out=b_qs2_s[:], in_=b_qs2[:, :])
            ident = wpool.tile([P, P], f32)
            make_identity(nc, ident[:])

            # ================= P1: layer-1 projections =================
            with tc.tile_pool(name="p1x", bufs=3) as p1x, \
                 tc.tile_pool(name="p1ps", bufs=4, space="PSUM") as p1ps, \
                 tc.tile_pool(name="p1o", bufs=4) as p1o:
                XB = 512
                for blk in range(NP_ // XB):
                    xT_s = p1x.tile([P, XB], f32, tag="xT")
                    nc.sync.dma_start(out=xT_s[:],
                                      in_=xT[:, blk * XB:(blk + 1) * XB])
                    for jj in range(XB // P):
                        t = blk * (XB // P) + jj
                        lhsT = xT_s[:, jj * P:(jj + 1) * P]
                        ps = p1ps.tile([P, 2 * D1], f32, tag="ps")
                        nc.tensor.matmul(out=ps[:], lhsT=lhsT, rhs=w_kv1_s[:],
                                         start=True, stop=biases_zero)
                        if not biases_zero:
                            nc.tensor.matmul(out=ps[:], lhsT=ones1[:],
                                             rhs=b_kv1_s[:], start=False, stop=True)
                        kv_o = p1o.tile([P, 2 * D1], f32, tag="kv")
                        nc.any.tensor_copy(out=kv_o[:], in_=ps[:])
                        nc.sync.dma_start(out=kv1_t[t * P:(t + 1) * P, :],
                                          in_=kv_o[:])
                        if t < per_core:   # own nodes (rotated order)
                            ps2 = p1ps.tile([P, 2 * D1], f32, tag="ps")
                            nc.tensor.matmul(out=ps2[:], lhsT=lhsT, rhs=w_qs1_s[:],
                                             start=True, stop=biases_zero)
                            if not biases_zero:
                                nc.tensor.matmul(out=ps2[:], lhsT=ones1[:],
                                                 rhs=b_qs1_s[:], start=False,
                                                 stop=True)
                            qs_o = p1o.tile([P, 2 * D1], f32, tag="kv")
                            nc.any.tensor_copy(out=qs_o[:], in_=ps2[:])
                            nc.sync.dma_start(out=qs1_t[t * P:(t + 1) * P, :],
                                              in_=qs_o[:])

            # ========== P2+P3: layer-1 attention + layer-2 projections ==========
            with tc.tile_pool(name="kvb", bufs=2) as kvb, \
                 tc.tile_pool(name="meta", bufs=2) as meta, \
                 tc.tile_pool(name="small", bufs=2) as small, \
                 tc.tile_pool(name="hps", bufs=2, space="PSUM") as hps, \
                 tc.tile_pool(name="houtp", bufs=2) as houtp:
                for j in range(per_core):
                    D = Ds_pos[j]
                    qs_s = meta.tile([P, 2 * D1], f32, tag="qs")
                    nc.sync.dma_start(out=qs_s[:],
                                      in_=qs1_t[j * P:(j + 1) * P, :])
                    idx_s = meta.tile([P, Dmax], i32, tag="idx")
                    nc.sync.dma_start(
                        out=idx_s[:, 0:D],
                        in_=idx1_f[slot_off[j]:slot_off[j + 1]]
                            .rearrange("(p d) -> p d", d=D))
                    bias_s = meta.tile([P, Dmax], f32, tag="bias")
                    nc.sync.dma_start(
                        out=bias_s[:, 0:D],
                        in_=bias_f[slot_off[j]:slot_off[j + 1]]
                            .rearrange("(p d) -> p d", d=D))
                    kv_s = kvb.tile([P, Dmax * 2 * D1], f32, tag="kv")
                    for d in range(D):
                        nc.gpsimd.indirect_dma_start(
                            out=kv_s[:, d * 2 * D1:(d + 1) * 2 * D1],
                            out_offset=None,
                            in_=kv1_t[:, :],
                            in_offset=bass.IndirectOffsetOnAxis(
                                ap=idx_s[:, d:d + 1], axis=0))
                    kv3 = kv_s[:].rearrange("p (d f) -> p d f", d=Dmax)
                    nc.vector.tensor_tensor(
                        out=kv3[:, 0:D, 0:D1], in0=kv3[:, 0:D, 0:D1],
                        in1=qs_s[:, 0:D1].unsqueeze(1).to_broadcast([P, D, D1]),
                        op=MUL)
                    lg = small.tile([P, Dmax * H1], f32, tag="lg")
                    lgv = lg[:, 0:D * H1].rearrange("p (d h) -> p d h", d=D)
                    nc.vector.reduce_sum(
                        out=lgv,
                        in_=kv3[:, 0:D, 0:D1].rearrange(
                            "p d (h c) -> p d h c", h=H1),
                        axis=X)
                    nc.vector.tensor_tensor(
                        out=lgv, in0=lgv,
                        in1=bias_s[:, 0:D].unsqueeze(2).to_broadcast([P, D, H1]),
                        op=ADD)
                    mx = small.tile([P, H1], f32, tag="mx")
                    nc.vector.reduce_max(
                        out=mx[:],
                        in_=lg[:, 0:D * H1].rearrange("p (d h) -> p h d", d=D),
                        axis=X)
                    nc.vector.tensor_tensor(
                        out=lgv, in0=lgv,
                        in1=mx[:].unsqueeze(1).to_broadcast([P, D, H1]),
                        op=SUB)
                    nc.scalar.activation(out=lg[:, 0:D * H1], in_=lg[:, 0:D * H1],
                                         func=EXP)
                    sm = small.tile([P, H1], f32, tag="sm")
                    nc.vector.reduce_sum(
                        out=sm[:],
                        in_=lg[:, 0:D * H1].rearrange("p (d h) -> p h d", d=D),
                        axis=X)
                    nc.vector.tensor_scalar_add(out=sm[:], in0=sm[:], scalar1=1e-16)
                    rc = small.tile([P, H1], f32, tag="rc")
                    nc.vector.reciprocal(out=rc[:], in_=sm[:])
                    nc.vector.tensor_tensor(
                        out=kv3[:, 0:D, D1:2 * D1].rearrange(
                            "p d (h c) -> p d h c", h=H1),
                        in0=kv3[:, 0:D, D1:2 * D1].rearrange(
                            "p d (h c) -> p d h c", h=H1),
                        in1=lgv.unsqueeze(3).to_broadcast([P, D, H1, C1]),
                        op=MUL)
                    att = houtp.tile([P, D1], f32, tag="att")
                    nc.vector.reduce_sum(
                        out=att[:],
                        in_=kv3[:, 0:D, D1:2 * D1].transpose([0, 2, 1]),
                        axis=X)
                    nc.vector.tensor_tensor(
                        out=att[:].rearrange("p (h c) -> p h c", h=H1),
                        in0=att[:].rearrange("p (h c) -> p h c", h=H1),
                        in1=rc[:].unsqueeze(2).to_broadcast([P, H1, C1]),
                        op=MUL)
                    nc.vector.tensor_add(out=att[:], in0=att[:],
                                         in1=qs_s[:, D1:2 * D1])
                    # ELU: h = relu(z) + exp(min(z,0)) - 1
                    zmin = houtp.tile([P, D1], f32, tag="zmin")
                    nc.vector.tensor_scalar_min(out=zmin[:], in0=att[:],
                                                scalar1=0.0)
                    nc.scalar.activation(out=zmin[:], in_=zmin[:], func=EXP)
                    h_s = houtp.tile([P, D1], f32, tag="h")
                    nc.scalar.activation(out=h_s[:], in_=att[:], func=RELU)
                    nc.vector.tensor_add(out=h_s[:], in0=h_s[:], in1=zmin[:])
                    nc.vector.tensor_scalar_add(out=h_s[:], in0=h_s[:],
                                                scalar1=-1.0)
                    nc.sync.dma_start(out=h_t[j * P:(j + 1) * P, :], in_=h_s[:])

                    # ---- layer-2 projections for this tile ----
                    hT0 = hps.tile([P, P], f32, tag="hT")
                    nc.tensor.transpose(out=hT0[:], in_=h_s[:, 0:P],
                                        identity=ident[:])
                    hT0s = houtp.tile([P, P], f32, tag="hT0s")
                    nc.any.tensor_copy(out=hT0s[:], in_=hT0[:])
                    hT1 = hps.tile([P, P], f32, tag="hT")
                    nc.tensor.transpose(out=hT1[:], in_=h_s[:, P:2 * P],
                                        identity=ident[:])
                    hT1s = houtp.tile([P, P], f32, tag="hT1s")
                    nc.any.tensor_copy(out=hT1s[:], in_=hT1[:])
                    for wi, (wt, dest) in enumerate(((w_kv2_s, kv2_own),
                                                     (w_qs2_s, qs2_t))):
                        ps = hps.tile([P, 2 * D2], f32, tag="ps2")
                        nc.tensor.matmul(out=ps[:], lhsT=hT0s[:],
                                         rhs=wt[:, 0:2 * D2],
                                         start=True, stop=False)
                        nc.tensor.matmul(out=ps[:], lhsT=hT1s[:],
                                         rhs=wt[:, 2 * D2:4 * D2],
                                         start=False, stop=biases_zero)
                        if not biases_zero:
                            bs = b_kv2_s if wi == 0 else b_qs2_s
                            nc.tensor.matmul(out=ps[:], lhsT=ones1[:], rhs=bs[:],
                                             start=False, stop=True)
                        os_ = houtp.tile([P, 2 * D2], f32, tag="os2")
                        nc.any.tensor_copy(out=os_[:], in_=ps[:])
                        nc.sync.dma_start(out=dest[j * P:(j + 1) * P, :],
                                          in_=os_[:])

            # ================= P4: AllGather kv2 =================
            if sim1:
                for c in range(N_CORES):
                    nc.sync.dma_start(
                        out=kv2_full[c * NOWN:(c + 1) * NOWN, :],
                        in_=kv2_own[:, :])
            else:
                nc.gpsimd.collective_compute(
                    "AllGather", mybir.AluOpType.bypass,
                    replica_groups=[list(range(N_CORES))],
                    ins=[kv2_own.ap().opt()],
                    outs=[kv2_full.ap().opt()],
                )

            # ================= P5: layer-2 attention =================
            with tc.tile_pool(name="kvb2", bufs=2) as kvb2, \
                 tc.tile_pool(name="meta2", bufs=2) as meta2, \
                 tc.tile_pool(name="small2", bufs=2) as small2, \
                 tc.tile_pool(name="outp", bufs=2) as outp:
                for j in range(per_core):
                    D = Ds_pos[j]
                    qs_s = meta2.tile([P, 2 * D2], f32, tag="qs2")
                    nc.sync.dma_start(out=qs_s[:],
                                      in_=qs2_t[j * P:(j + 1) * P, :])
                    idx_s = meta2.tile([P, Dmax], i32, tag="idx2")
                    nc.sync.dma_start(
                        out=idx_s[:, 0:D],
                        in_=idx2_f[slot_off[j]:slot_off[j + 1]]
                            .rearrange("(p d) -> p d", d=D))
                    bias_s = meta2.tile([P, Dmax], f32, tag="bias2")
                    nc.sync.dma_start(
                        out=bias_s[:, 0:D],
                        in_=bias_f[slot_off[j]:slot_off[j + 1]]
                            .rearrange("(p d) -> p d", d=D))
                    kv_s = kvb2.tile([P, Dmax * 2 * D2], f32, tag="kv2")
                    for d in range(D):
                        nc.gpsimd.indirect_dma_start(
                            out=kv_s[:, d * 2 * D2:(d + 1) * 2 * D2],
                            out_offset=None,
                            in_=kv2_full[:, :],
                            in_offset=bass.IndirectOffsetOnAxis(
                                ap=idx_s[:, d:d + 1], axis=0))
                    kv3 = kv_s[:].rearrange("p (d f) -> p d f", d=Dmax)
                    nc.vector.tensor_tensor(
                        out=kv3[:, 0:D, 0:D2], in0=kv3[:, 0:D, 0:D2],
                        in1=qs_s[:, 0:D2].unsqueeze(1).to_broadcast([P, D, D2]),
                        op=MUL)
                    lg = small2.tile([P, Dmax], f32, tag="lg2")
                    nc.vector.reduce_sum(out=lg[:, 0:D], in_=kv3[:, 0:D, 0:D2],
                                         axis=X)
                    nc.vector.tensor_add(out=lg[:, 0:D], in0=lg[:, 0:D],
                                         in1=bias_s[:, 0:D])
                    mx = small2.tile([P, 1], f32, tag="mx2")
                    nc.vector.reduce_max(out=mx[:], in_=lg[:, 0:D], axis=X)
                    nc.vector.tensor_tensor(out=lg[:, 0:D], in0=lg[:, 0:D],
                                            in1=mx[:].to_broadcast([P, D]),
                                            op=SUB)
                    nc.scalar.activation(out=lg[:, 0:D], in_=lg[:, 0:D], func=EXP)
                    sm = small2.tile([P, 1], f32, tag="sm2")
                    nc.vector.reduce_sum(out=sm[:], in_=lg[:, 0:D], axis=X)
                    nc.vector.tensor_scalar_add(out=sm[:], in0=sm[:],
                                                scalar1=1e-16)
                    rc = small2.tile([P, 1], f32, tag="rc2")
                    nc.vector.reciprocal(out=rc[:], in_=sm[:])
                    nc.vector.tensor_tensor(
                        out=kv3[:, 0:D, D2:2 * D2], in0=kv3[:, 0:D, D2:2 * D2],
                        in1=lg[:, 0:D].unsqueeze(2).to_broadcast([P, D, D2]),
                        op=MUL)
                    att = outp.tile([P, D2], f32, tag="att2")
                    nc.vector.reduce_sum(
                        out=att[:],
                        in_=kv3[:, 0:D, D2:2 * D2].transpose([0, 2, 1]),
                        axis=X)
                    nc.vector.tensor_tensor(out=att[:], in0=att[:],
                                            in1=rc[:].to_broadcast([P, D2]),
                                            op=MUL)
                    nc.vector.tensor_add(out=att[:], in0=att[:],
                                         in1=qs_s[:, D2:2 * D2])
                    nc.sync.dma_start(out=out_d[j * P:(j + 1) * P, :],
                                      in_=att[:])

    nc.compile()
    return nc


_CACHE = {}


def _get_program(NP_, per_core, Ds_pos, biases_zero):
    key = (NP_, per_core, tuple(Ds_pos), biases_zero)
    if key not in _CACHE:
        _CACHE[key] = _build_program(NP_, per_core, Ds_pos, biases_zero)
    return _CACHE[key]


def kernel(**inputs):
    from concourse.bass_utils import run_bass_kernel_spmd

    x = np.asarray(inputs["x"], np.float32)
    edge_index = np.asarray(inputs["edge_index"])
    plan = _plan(edge_index)
    NP_ = plan["NP"]
    per_core = plan["per_core"]
    Ds = plan["Ds"]
    NOWN = per_core * P

    # position-aligned degrees (SPMD: one program for all cores)
    Ds_pos = [max(Ds[c * per_core + j] for c in range(N_CORES))
              for j in range(per_core)]

    s1 = 1.0 / np.sqrt(np.float32(C1))
    s2 = 1.0 / np.sqrt(np.float32(D2))
    w_kv1 = np.ascontiguousarray(
        np.concatenate([inputs["w1k"], inputs["w1v"]], axis=1), np.float32)
    w_qs1 = np.ascontiguousarray(
        np.concatenate([np.asarray(inputs["w1q"]) * s1, inputs["w1s"]], axis=1),
        np.float32)
    w_kv2 = np.ascontiguousarray(
        np.concatenate([inputs["w2k"], inputs["w2v"]], axis=1), np.float32)
    w_qs2 = np.ascontiguousarray(
        np.concatenate([np.asarray(inputs["w2q"]) * s2, inputs["w2s"]], axis=1),
        np.float32)
    b_kv1 = np.ascontiguousarray(
        np.concatenate([inputs["b1k"], inputs["b1v"]])[None], np.float32)
    b_qs1 = np.ascontiguousarray(
        np.concatenate([np.asarray(inputs["b1q"]) * s1, inputs["b1s"]])[None],
        np.float32)
    b_kv2 = np.ascontiguousarray(
        np.concatenate([inputs["b2k"], inputs["b2v"]])[None], np.float32)
    b_qs2 = np.ascontiguousarray(
        np.concatenate([np.asarray(inputs["b2q"]) * s2, inputs["b2s"]])[None],
        np.float32)
    biases_zero = all(not np.any(b) for b in (b_kv1, b_qs1, b_kv2, b_qs2))

    nc = _get_program(NP_, per_core, Ds_pos, biases_zero)

    xpad = np.concatenate([x, np.zeros((NP_ - N, IN_DIM), np.float32)])
    x_new = xpad[plan["perm"]]
    xT_new = np.ascontiguousarray(x_new.T)

    in_maps = []
    for c in range(N_CORES):
        own0 = c * NOWN
        rot = np.concatenate([np.arange(own0, own0 + NOWN),
                              np.arange(0, own0),
                              np.arange(own0 + NOWN, NP_)])
        inv_rot = np.empty(NP_, np.int64)
        inv_rot[rot] = np.arange(NP_)
        xT_c = np.ascontiguousarray(xT_new[:, rot])
        idx1_list, idx2_list, bias_list = [], [], []
        for j in range(per_core):
            t_new = c * per_core + j
            D = Ds[t_new]
            Dp = Ds_pos[j]
            it = plan["idx_tiles"][t_new]       # [P, D] new ids
            bt = plan["bias_tiles"][t_new]
            i1 = np.zeros((P, Dp), np.int32)
            i2 = np.zeros((P, Dp), np.int32)
            bp = np.full((P, Dp), NEG, np.float32)
            i1[:, :D] = inv_rot[it]             # rotated ids (layer-1 table)
            i2[:, :D] = it                      # global new ids (layer-2 table)
            bp[:, :D] = bt
            idx1_list.append(i1.reshape(-1))
            idx2_list.append(i2.reshape(-1))
            bias_list.append(bp.reshape(-1))
        in_maps.append(dict(
            xT=xT_c,
            w_kv1=w_kv1, w_qs1=w_qs1, w_kv2=w_kv2, w_qs2=w_qs2,
            b_kv1=b_kv1, b_qs1=b_qs1, b_kv2=b_kv2, b_qs2=b_qs2,
            idx1_f=np.concatenate(idx1_list),
            idx2_f=np.concatenate(idx2_list),
            bias_f=np.concatenate(bias_list),
        ))

    res = run_bass_kernel_spmd(nc, in_maps, core_ids=list(range(N_CORES)))
    kernel.last_results = res

    out_new = np.concatenate([np.asarray(res.results[c]["out"])
                              for c in range(N_CORES)])
    mask = plan["perm"] < N
    out = np.empty((N, D2), np.float32)
    out[plan["perm"][mask]] = out_new[mask]
    return out



# revision 3
# speedup vs baseline: 2.1304x; 2.1304x over previous
"""GraphTransformer 2-layer (TransformerConv x2) on 8 Trainium2 NeuronCores.

Sharding: destination-node partitioning with degree-sorted padded tiles.
  - Pad N=50000 -> N'=50176 (392 tiles of 128 nodes). Sort nodes by
    in-degree, bin-pack the tiles onto 8 cores (49 each, balancing slots).
  - Each core receives x^T (bf16) in a per-core *rotated* node order (its
    own nodes first), computes the full layer-1 K|V table [N',512] bf16
    on-device, plus Q|S for its own nodes (kept in SBUF).
  - Attention per dst-tile: ONE indirect DMA gathers all D neighbors'
    K|V rows ([P, D] offset AP).  Features are stored c-major ([c,h])
    so every big elementwise op keeps a packed 2-byte last dim and runs
    in the DVE 2x mode; reductions are tree-adds (TensorReduce has no
    16-bit fast path).  Padding slots point at row 0 with a -1e30 logit
    bias, so exp() kills them without a max-subtraction pass.
  - Layer-2 K|V [N',20] fp32 is computed from the local h chunk and
    AllGathered (4 MB on the wire).
All indices/degrees/tile shapes are baked in at build time from the actual
inputs. kernel() builds + runs the single-launch SPMD program and
unpermutes the output on the host.
"""

import numpy as np

N_CORES = 8
N = 50000
IN_DIM = 128
D1 = 256            # heads*hid layer1
H1, C1 = 8, 32
D2 = 10             # layer2 out channels (1 head)
P = 128
NEG = -1.0e30


def _plan(edge_index):
    src = np.asarray(edge_index[0], dtype=np.int64)
    dst = np.asarray(edge_index[1], dtype=np.int64)
    deg = np.bincount(dst, minlength=N)
    NP_ = ((N + N_CORES * P - 1) // (N_CORES * P)) * (N_CORES * P)  # 50176
    n_tiles = NP_ // P                                              # 392
    per_core = n_tiles // N_CORES                                   # 49

    degp = np.concatenate([deg, np.zeros(NP_ - N, np.int64)])
    order0 = np.argsort(degp, kind="stable")        # old(padded) ids, deg asc
    tile_of = order0.reshape(n_tiles, P)            # prelim tile -> old ids
    tile_D = degp[tile_of].max(axis=1)

    # bin-pack tiles onto cores: largest-first greedy with capacity
    t_order = np.argsort(-tile_D, kind="stable")
    loads = np.zeros(N_CORES, np.int64)
    counts = np.zeros(N_CORES, np.int64)
    assign = [[] for _ in range(N_CORES)]
    for t in t_order:
        open_cores = [c for c in range(N_CORES) if counts[c] < per_core]
        c = min(open_cores, key=lambda cc: (loads[cc], cc))
        assign[c].append(int(t))
        loads[c] += int(tile_D[t])
        counts[c] += 1
    for c in range(N_CORES):
        assign[c].sort(key=lambda t: int(tile_D[t]))

    final_tiles = [t for c in range(N_CORES) for t in assign[c]]
    perm = tile_of[final_tiles].reshape(-1)         # new id -> old(padded) id
    inv = np.empty(NP_, np.int64)
    inv[perm] = np.arange(NP_)

    Ds = degp[perm].reshape(n_tiles, P).max(axis=1).astype(np.int64)

    # per-(new)tile neighbor tables in NEW ids; pad idx=0, bias=NEG
    dst_new = inv[dst]
    src_new = inv[src]
    eo = np.argsort(dst_new, kind="stable")
    dst_s = dst_new[eo]
    src_s = src_new[eo]
    row_start = np.searchsorted(dst_s, np.arange(NP_))
    row_end = np.searchsorted(dst_s, np.arange(NP_) + 1)

    idx_tiles, bias_tiles = [], []
    for t in range(n_tiles):
        D = int(Ds[t])
        it = np.zeros((P, D), np.int64)
        bt = np.full((P, D), NEG, np.float32)
        for p in range(P):
            s, e = row_start[t * P + p], row_end[t * P + p]
            k = e - s
            it[p, :k] = src_s[s:e]
            bt[p, :k] = 0.0
        idx_tiles.append(it)
        bias_tiles.append(bt)

    return dict(NP=NP_, n_tiles=n_tiles, per_core=per_core, perm=perm,
                inv=inv, Ds=[int(d) for d in Ds], idx_tiles=idx_tiles,
                bias_tiles=bias_tiles)


def _build_program(NP_, per_core, Ds_pos, biases_zero, sim1=False):
    import concourse.bass as bass
    import concourse.mybir as mybir
    from concourse import bacc
    from concourse.tile import TileContext
    from concourse.masks import make_identity

    f32 = mybir.dt.float32
    bf16 = mybir.dt.bfloat16
    i32 = mybir.dt.int32
    NOWN = per_core * P
    Dmax = max(Ds_pos)
    totD = sum(Ds_pos)
    cum = [0]
    for d in Ds_pos:
        cum.append(cum[-1] + d)

    nc = bacc.Bacc("TRN2", target_bir_lowering=False, debug=False,
                   num_devices=1 if sim1 else N_CORES)

    xT = nc.dram_tensor("xT", [IN_DIM, NP_], bf16, kind="ExternalInput")
    w_kv1 = nc.dram_tensor("w_kv1", [IN_DIM, 2 * D1], bf16, kind="ExternalInput")
    w_qs1 = nc.dram_tensor("w_qs1", [IN_DIM, 2 * D1], bf16, kind="ExternalInput")
    # layer-2 weights packed as [kv2_c0 | qs2_c0] ; [kv2_c1 | qs2_c1]
    w2a = nc.dram_tensor("w2a", [P, 2 * (2 * D2)], bf16, kind="ExternalInput")
    w2b = nc.dram_tensor("w2b", [P, 2 * (2 * D2)], bf16, kind="ExternalInput")
    b_kv1 = nc.dram_tensor("b_kv1", [1, 2 * D1], bf16, kind="ExternalInput")
    b_qs1 = nc.dram_tensor("b_qs1", [1, 2 * D1], bf16, kind="ExternalInput")
    b_kv2 = nc.dram_tensor("b_kv2", [1, 2 * D2], bf16, kind="ExternalInput")
    b_qs2 = nc.dram_tensor("b_qs2", [1, 2 * D2], bf16, kind="ExternalInput")
    idx1_pm = nc.dram_tensor("idx1_pm", [P, totD], i32, kind="ExternalInput")
    idx2_pm = nc.dram_tensor("idx2_pm", [P, totD], i32, kind="ExternalInput")
    bias_pm = nc.dram_tensor("bias_pm", [P, totD], bf16, kind="ExternalInput")
    out_d = nc.dram_tensor("out", [NOWN, D2], f32, kind="ExternalOutput")

    kv1_t = nc.dram_tensor("kv1_t", [NP_, 2 * D1], bf16)
    kv2_own = nc.dram_tensor("kv2_own", [NOWN, 2 * D2], f32)
    kv2_full = nc.dram_tensor("kv2_full", [NP_, 2 * D2], f32, addr_space="Shared")

    X = mybir.AxisListType.X
    MUL = mybir.AluOpType.mult
    ADD = mybir.AluOpType.add
    SUB = mybir.AluOpType.subtract
    EXP = mybir.ActivationFunctionType.Exp

    with nc.allow_low_precision(reason="bf16 attention within 2e-2 tolerance"), \
         TileContext(nc) as tc:
        with tc.tile_pool(name="wpool", bufs=1) as wpool:
            w_kv1_s = wpool.tile([IN_DIM, 2 * D1], bf16)
            nc.sync.dma_start(out=w_kv1_s[:], in_=w_kv1[:, :])
            w_qs1_s = wpool.tile([IN_DIM, 2 * D1], bf16)
            nc.sync.dma_start(out=w_qs1_s[:], in_=w_qs1[:, :])
            w2a_s = wpool.tile([P, 2 * (2 * D2)], bf16)
            nc.sync.dma_start(out=w2a_s[:], in_=w2a[:, :])
            w2b_s = wpool.tile([P, 2 * (2 * D2)], bf16)
            nc.sync.dma_start(out=w2b_s[:], in_=w2b[:, :])
            idx1_sb = wpool.tile([P, totD], i32)
            nc.sync.dma_start(out=idx1_sb[:], in_=idx1_pm[:, :])
            idx2_sb = wpool.tile([P, totD], i32)
            nc.sync.dma_start(out=idx2_sb[:], in_=idx2_pm[:, :])
            biasm_sb = wpool.tile([P, totD], bf16)
            nc.sync.dma_start(out=biasm_sb[:], in_=bias_pm[:, :])
            if not biases_zero:
                ones1 = wpool.tile([1, P], bf16)
                nc.vector.memset(ones1[:], 1.0)
                b_kv1_s = wpool.tile([1, 2 * D1], bf16)
                nc.sync.dma_start(out=b_kv1_s[:], in_=b_kv1[:, :])
                b_qs1_s = wpool.tile([1, 2 * D1], bf16)
                nc.sync.dma_start(out=b_qs1_s[:], in_=b_qs1[:, :])
                b_kv2_s = wpool.tile([1, 2 * D2], bf16)
                nc.sync.dma_start(out=b_kv2_s[:], in_=b_kv2[:, :])
                b_qs2_s = wpool.tile([1, 2 * D2], bf16)
                nc.sync.dma_start(out=b_qs2_s[:], in_=b_qs2[:, :])
            ident = wpool.tile([P, P], bf16)
            make_identity(nc, ident[:])
            # SBUF-resident per-core state
            qs1_sb = wpool.tile([P, per_core * 2 * D1], bf16)   # Q|S layer1
            kv2_sb = wpool.tile([P, per_core * 2 * D2], f32)    # K|V layer2
            qs2_sb = wpool.tile([P, per_core * 2 * D2], f32)    # Q|S layer2
            out_sb = wpool.tile([P, per_core * D2], f32)        # final out

            copy_engines = [nc.vector, nc.scalar, nc.gpsimd]

            # ================= P1: layer-1 projections =================
            TB = 4                     # tiles per x block
            XB = TB * P                # 512 columns per load
            with tc.tile_pool(name="p1x", bufs=3) as p1x, \
                 tc.tile_pool(name="p1ps", bufs=4, space="PSUM") as p1ps, \
                 tc.tile_pool(name="p1o", bufs=3) as p1o:
                for blk in range(NP_ // XB):
                    xT_s = p1x.tile([P, XB], bf16, tag="xT")
                    nc.sync.dma_start(out=xT_s[:],
                                      in_=xT[:, blk * XB:(blk + 1) * XB])
                    kvstage = p1o.tile([P, TB * 2 * D1], bf16, tag="kv")
                    for jj in range(TB):
                        t = blk * TB + jj
                        lhsT = xT_s[:, jj * P:(jj + 1) * P]
                        ps = p1ps.tile([P, 2 * D1], f32, tag="ps")
                        nc.tensor.matmul(out=ps[:], lhsT=lhsT, rhs=w_kv1_s[:],
                                         start=True, stop=biases_zero)
                        if not biases_zero:
                            nc.tensor.matmul(out=ps[:], lhsT=ones1[:],
                                             rhs=b_kv1_s[:], start=False, stop=True)
                        eng = copy_engines[t % 3]
                        if eng is nc.scalar:
                            eng.copy(out=kvstage[:, jj * 2 * D1:(jj + 1) * 2 * D1],
                                     in_=ps[:])
                        else:
                            eng.tensor_copy(
                                out=kvstage[:, jj * 2 * D1:(jj + 1) * 2 * D1],
                                in_=ps[:])
                        if t < per_core:   # own nodes (rotated order)
                            ps2 = p1ps.tile([P, 2 * D1], f32, tag="ps")
                            nc.tensor.matmul(out=ps2[:], lhsT=lhsT, rhs=w_qs1_s[:],
                                             start=True, stop=biases_zero)
                            if not biases_zero:
                                nc.tensor.matmul(out=ps2[:], lhsT=ones1[:],
                                                 rhs=b_qs1_s[:], start=False,
                                                 stop=True)
                            eng2 = copy_engines[(t + 1) % 3]
                            dst_ap = qs1_sb[:, t * 2 * D1:(t + 1) * 2 * D1]
                            if eng2 is nc.scalar:
                                eng2.copy(out=dst_ap, in_=ps2[:])
                            else:
                                eng2.tensor_copy(out=dst_ap, in_=ps2[:])
                    nc.sync.dma_start(
                        out=kv1_t[blk * XB:(blk + 1) * XB, :]
                            .rearrange("(t p) f -> p t f", p=P),
                        in_=kvstage[:].rearrange("p (t f) -> p t f", t=TB))

            # ========== P2: layer-1 attention + layer-2 projections ==========
            with tc.tile_pool(name="kvb", bufs=2) as kvb, \
                 tc.tile_pool(name="prodb", bufs=2) as prodb, \
                 tc.tile_pool(name="small", bufs=3) as small, \
                 tc.tile_pool(name="hps", bufs=2, space="PSUM") as hps, \
                 tc.tile_pool(name="hps2", bufs=2, space="PSUM") as hps2, \
                 tc.tile_pool(name="houtp", bufs=2) as houtp:
                for j in range(per_core):
                    D = Ds_pos[j]
                    c0 = cum[j]
                    kv_s = kvb.tile([P, Dmax * 2 * D1], bf16, tag="kv")
                    nc.gpsimd.indirect_dma_start(
                        out=kv_s[:, 0:D * 2 * D1],
                        out_offset=None,
                        in_=kv1_t[:, :],
                        in_offset=bass.IndirectOffsetOnAxis(
                            ap=idx1_sb[:, c0:c0 + D], axis=0))
                    kv5 = kv_s[:, 0:D * 2 * D1].rearrange(
                        "p (d x c h) -> p d x c h", d=D, x=2, c=C1)
                    K4 = kv5[:, :, 0, :, :]              # [P, D, 32, 8] bf16
                    V4 = kv5[:, :, 1, :, :]
                    V3 = kv_s[:, 0:D * 2 * D1].rearrange(
                        "p (d x f) -> p d x f", d=D, x=2)[:, :, 1, :]  # [P,D,256]
                    q3 = qs1_sb[:, j * 2 * D1:j * 2 * D1 + D1].rearrange(
                        "p (c h) -> p c h", c=C1)
                    prod = prodb.tile([P, Dmax * D1], bf16, tag="prod")
                    P4 = prod[:, 0:D * D1].rearrange(
                        "p (d c h) -> p d c h", d=D, c=C1)
                    # logits: q . k  (bf16 2x mode: packed last dim = heads)
                    nc.vector.tensor_tensor(
                        out=P4, in0=K4,
                        in1=q3.unsqueeze(1).to_broadcast([P, D, C1, H1]),
                        op=MUL)
                    cur = C1
                    while cur > 1:
                        nh = cur // 2
                        nc.vector.tensor_tensor(
                            out=P4[:, :, 0:nh, :], in0=P4[:, :, 0:nh, :],
                            in1=P4[:, :, cur - nh:cur, :], op=ADD)
                        cur = nh
                    # finalize logits in fp32 with padding bias
                    lg = small.tile([P, Dmax * H1], f32, tag="lg")
                    L3 = lg[:, 0:D * H1].rearrange("p (d h) -> p d h", d=D)
                    nc.vector.tensor_tensor(
                        out=L3, in0=P4[:, :, 0, :],
                        in1=biasm_sb[:, c0:c0 + D].unsqueeze(2)
                            .to_broadcast([P, D, H1]),
                        op=ADD)
                    # softmax (no max-subtraction: logits are O(10) bounded;
                    # pad slots have -1e30 -> exp == 0)
                    e_bf = small.tile([P, Dmax * H1], bf16, tag="ebf")
                    nc.scalar.activation(out=e_bf[:, 0:D * H1],
                                         in_=lg[:, 0:D * H1], func=EXP)
                    sm = small.tile([P, H1], f32, tag="sm")
                    nc.vector.reduce_sum(
                        out=sm[:],
                        in_=e_bf[:, 0:D * H1].rearrange("p (d h) -> p h d", d=D),
                        axis=X)
                    nc.gpsimd.tensor_scalar_add(out=sm[:], in0=sm[:],
                                                scalar1=1e-16)
                    rc = small.tile([P, H1], f32, tag="rc")
                    nc.vector.reciprocal(out=rc[:], in_=sm[:])
                    rcb = small.tile([P, H1], bf16, tag="rcb")
                    nc.gpsimd.tensor_copy(out=rcb[:], in_=rc[:])
                    # weighted V: V *= e  (bf16 2x), then tree-sum over d
                    E3 = e_bf[:, 0:D * H1].rearrange("p (d h) -> p d h", d=D)
                    nc.vector.tensor_tensor(
                        out=V4, in0=V4,
                        in1=E3.unsqueeze(2).to_broadcast([P, D, C1, H1]),
                        op=MUL)
                    cur = D
                    while cur > 1:
                        nh = cur // 2
                        nc.vector.tensor_tensor(
                            out=V3[:, 0:nh, :], in0=V3[:, 0:nh, :],
                            in1=V3[:, cur - nh:cur, :], op=ADD)
                        cur = cur - nh
                    att = houtp.tile([P, D1], f32, tag="att")
                    A3 = att[:].rearrange("p (c h) -> p c h", c=C1)
                    nc.vector.tensor_tensor(
                        out=A3,
                        in0=V3[:, 0, :].rearrange("p (c h) -> p c h", c=C1),
                        in1=rcb[:].unsqueeze(1).to_broadcast([P, C1, H1]),
                        op=MUL)
                    nc.vector.tensor_tensor(
                        out=att[:], in0=att[:],
                        in1=qs1_sb[:, j * 2 * D1 + D1:(j + 1) * 2 * D1],
                        op=ADD)
                    # ELU: h = relu(z) + exp(min(z,0)) - 1
                    zmin = houtp.tile([P, D1], f32, tag="zmin")
                    nc.vector.tensor_scalar_min(out=zmin[:], in0=att[:],
                                                scalar1=0.0)
                    ez = houtp.tile([P, D1], f32, tag="ez")
                    nc.scalar.activation(out=ez[:], in_=zmin[:], func=EXP)
                    nc.vector.tensor_sub(out=att[:], in0=att[:], in1=zmin[:])
                    nc.vector.tensor_add(out=att[:], in0=att[:], in1=ez[:])
                    h_bf = houtp.tile([P, D1], bf16, tag="h")
                    nc.vector.tensor_scalar_add(out=h_bf[:], in0=att[:],
                                                scalar1=-1.0)

                    # ---- layer-2 projections for this tile ----
                    hT0 = hps.tile([P, P], bf16, tag="hT")
                    nc.tensor.transpose(out=hT0[:], in_=h_bf[:, 0:P],
                                        identity=ident[:])
                    hT0s = houtp.tile([P, P], bf16, tag="hT0s")
                    nc.scalar.copy(out=hT0s[:], in_=hT0[:])
                    hT1 = hps.tile([P, P], bf16, tag="hT")
                    nc.tensor.transpose(out=hT1[:], in_=h_bf[:, P:2 * P],
                                        identity=ident[:])
                    hT1s = houtp.tile([P, P], bf16, tag="hT1s")
                    nc.scalar.copy(out=hT1s[:], in_=hT1[:])
                    ps2 = hps2.tile([P, 2 * (2 * D2)], f32, tag="ps2")
                    nc.tensor.matmul(out=ps2[:], lhsT=hT0s[:], rhs=w2a_s[:],
                                     start=True, stop=False)
                    nc.tensor.matmul(out=ps2[:], lhsT=hT1s[:], rhs=w2b_s[:],
                                     start=False, stop=biases_zero)
                    if not biases_zero:
                        nc.tensor.matmul(
                            out=ps2[:, 0:2 * D2], lhsT=ones1[:],
                            rhs=b_kv2_s[:], start=False, stop=False)
                        nc.tensor.matmul(
                            out=ps2[:, 2 * D2:4 * D2], lhsT=ones1[:],
                            rhs=b_qs2_s[:], start=False, stop=True)
                    nc.scalar.copy(out=kv2_sb[:, j * 2 * D2:(j + 1) * 2 * D2],
                                   in_=ps2[:, 0:2 * D2])
                    nc.scalar.copy(out=qs2_sb[:, j * 2 * D2:(j + 1) * 2 * D2],
                                   in_=ps2[:, 2 * D2:4 * D2])
                nc.sync.dma_start(
                    out=kv2_own[:, :].rearrange("(t p) f -> p t f", p=P),
                    in_=kv2_sb[:].rearrange("p (t f) -> p t f", t=per_core))

            # ================= P4: AllGather kv2 =================
            if sim1:
                for c in range(N_CORES):
                    nc.sync.dma_start(
                        out=kv2_full[c * NOWN:(c + 1) * NOWN, :],
                        in_=kv2_own[:, :])
            else:
                nc.gpsimd.collective_compute(
                    "AllGather", mybir.AluOpType.bypass,
                    replica_groups=[list(range(N_CORES))],
                    ins=[kv2_own.ap().opt()],
                    outs=[kv2_full.ap().opt()],
                )

            # ================= P5: layer-2 attention =================
            with tc.tile_pool(name="kvb2", bufs=2) as kvb2, \
                 tc.tile_pool(name="small2", bufs=3) as small2:
                for j in range(per_core):
                    D = Ds_pos[j]
                    c0 = cum[j]
                    kv2g = kvb2.tile([P, Dmax * 2 * D2], f32, tag="kv2")
                    nc.gpsimd.indirect_dma_start(
                        out=kv2g[:, 0:D * 2 * D2],
                        out_offset=None,
                        in_=kv2_full[:, :],
                        in_offset=bass.IndirectOffsetOnAxis(
                            ap=idx2_sb[:, c0:c0 + D], axis=0))
                    G3 = kv2g[:, 0:D * 2 * D2].rearrange(
                        "p (d f) -> p d f", d=D)
                    prod2 = small2.tile([P, Dmax * D2], f32, tag="p2")
                    P3 = prod2[:, 0:D * D2].rearrange("p (d f) -> p d f", d=D)
                    nc.vector.tensor_tensor(
                        out=P3, in0=G3[:, :, 0:D2],
                        in1=qs2_sb[:, j * 2 * D2:j * 2 * D2 + D2]
                            .unsqueeze(1).to_broadcast([P, D, D2]),
                        op=MUL)
                    lg2 = small2.tile([P, Dmax], f32, tag="lg2")
                    nc.vector.reduce_sum(out=lg2[:, 0:D], in_=P3, axis=X)
                    nc.vector.tensor_tensor(out=lg2[:, 0:D], in0=lg2[:, 0:D],
                                            in1=biasm_sb[:, c0:c0 + D], op=ADD)
                    e2 = small2.tile([P, Dmax], f32, tag="e2")
                    nc.scalar.activation(out=e2[:, 0:D], in_=lg2[:, 0:D],
                                         func=EXP)
                    sm2 = small2.tile([P, 1], f32, tag="sm2")
                    nc.vector.reduce_sum(out=sm2[:], in_=e2[:, 0:D], axis=X)
                    nc.gpsimd.tensor_scalar_add(out=sm2[:], in0=sm2[:],
                                                scalar1=1e-16)
                    rc2 = small2.tile([P, 1], f32, tag="rc2")
                    nc.vector.reciprocal(out=rc2[:], in_=sm2[:])
                    nc.vector.tensor_tensor(
                        out=G3[:, :, D2:2 * D2], in0=G3[:, :, D2:2 * D2],
                        in1=e2[:, 0:D].unsqueeze(2).to_broadcast([P, D, D2]),
                        op=MUL)
                    att2 = small2.tile([P, D2], f32, tag="att2")
                    nc.vector.reduce_sum(
                        out=att2[:],
                        in_=G3[:, :, D2:2 * D2].transpose([0, 2, 1]),
                        axis=X)
                    nc.vector.tensor_scalar_mul(out=att2[:], in0=att2[:],
                                                scalar1=rc2[:])
                    nc.vector.tensor_tensor(
                        out=out_sb[:, j * D2:(j + 1) * D2], in0=att2[:],
                        in1=qs2_sb[:, j * 2 * D2 + D2:(j + 1) * 2 * D2],
                        op=ADD)
                nc.sync.dma_start(
                    out=out_d[:, :].rearrange("(t p) f -> p t f", p=P),
                    in_=out_sb[:].rearrange("p (t f) -> p t f", t=per_core))

    nc.compile()
    return nc


_CACHE = {}


def _get_program(NP_, per_core, Ds_pos, biases_zero):
    key = (NP_, per_core, tuple(Ds_pos), biases_zero)
    if key not in _CACHE:
        _CACHE[key] = _build_program(NP_, per_core, Ds_pos, biases_zero)
    return _CACHE[key]


def _cmajor_cols():
    # new column (c*8+h) <- old column (h*32+c)
    return (np.arange(D1).reshape(H1, C1)).T.reshape(-1)


def kernel(**inputs):
    import ml_dtypes
    from concourse.bass_utils import run_bass_kernel_spmd

    bf = ml_dtypes.bfloat16
    x = np.asarray(inputs["x"], np.float32)
    edge_index = np.asarray(inputs["edge_index"])
    plan = _plan(edge_index)
    NP_ = plan["NP"]
    per_core = plan["per_core"]
    Ds = plan["Ds"]
    NOWN = per_core * P

    # position-aligned degrees (SPMD: one program for all cores)
    Ds_pos = [max(Ds[c * per_core + j] for c in range(N_CORES))
              for j in range(per_core)]
    totD = sum(Ds_pos)
    cum = np.zeros(per_core + 1, np.int64)
    cum[1:] = np.cumsum(Ds_pos)

    cm = _cmajor_cols()
    s1 = 1.0 / np.sqrt(np.float32(C1))
    s2 = 1.0 / np.sqrt(np.float32(D2))
    w1k = np.asarray(inputs["w1k"], np.float32)[:, cm]
    w1v = np.asarray(inputs["w1v"], np.float32)[:, cm]
    w1q = np.asarray(inputs["w1q"], np.float32)[:, cm] * s1
    w1s = np.asarray(inputs["w1s"], np.float32)[:, cm]
    w_kv1 = np.ascontiguousarray(
        np.concatenate([w1k, w1v], axis=1)).astype(bf)
    w_qs1 = np.ascontiguousarray(
        np.concatenate([w1q, w1s], axis=1)).astype(bf)
    # layer-2 weights: rows permuted to c-major (h is c-major), packed as
    # [kv2 | qs2] per 128-row chunk
    w2k = np.asarray(inputs["w2k"], np.float32)[cm, :] * 1.0
    w2v = np.asarray(inputs["w2v"], np.float32)[cm, :]
    w2q = np.asarray(inputs["w2q"], np.float32)[cm, :] * s2
    w2s = np.asarray(inputs["w2s"], np.float32)[cm, :]
    wkv2 = np.concatenate([w2k, w2v], axis=1)      # [256, 20]
    wqs2 = np.concatenate([w2q, w2s], axis=1)      # [256, 20]
    w2a = np.ascontiguousarray(
        np.concatenate([wkv2[0:P], wqs2[0:P]], axis=1)).astype(bf)
    w2b = np.ascontiguousarray(
        np.concatenate([wkv2[P:2 * P], wqs2[P:2 * P]], axis=1)).astype(bf)
    b_kv1 = np.concatenate([np.asarray(inputs["b1k"], np.float32)[cm],
                            np.asarray(inputs["b1v"], np.float32)[cm]])[None]
    b_qs1 = np.concatenate([np.asarray(inputs["b1q"], np.float32)[cm] * s1,
                            np.asarray(inputs["b1s"], np.float32)[cm]])[None]
    b_kv2 = np.concatenate([np.asarray(inputs["b2k"], np.float32),
                            np.asarray(inputs["b2v"], np.float32)])[None]
    b_qs2 = np.concatenate([np.asarray(inputs["b2q"], np.float32) * s2,
                            np.asarray(inputs["b2s"], np.float32)])[None]
    biases_zero = all(not np.any(b) for b in (b_kv1, b_qs1, b_kv2, b_qs2))

    nc = _get_program(NP_, per_core, Ds_pos, biases_zero)

    xpad = np.concatenate([x, np.zeros((NP_ - N, IN_DIM), np.float32)])
    x_new = xpad[plan["perm"]]
    xT_new = np.ascontiguousarray(x_new.T).astype(bf)

    in_maps = []
    for c in range(N_CORES):
        own0 = c * NOWN
        rot = np.concatenate([np.arange(own0, own0 + NOWN),
                              np.arange(0, own0),
                              np.arange(own0 + NOWN, NP_)])
        inv_rot = np.empty(NP_, np.int64)
        inv_rot[rot] = np.arange(NP_)
        xT_c = np.ascontiguousarray(xT_new[:, rot])
        idx1 = np.zeros((P, totD), np.int32)
        idx2 = np.zeros((P, totD), np.int32)
        biasm = np.full((P, totD), NEG, np.float32)
        for j in range(per_core):
            t_new = c * per_core + j
            D = Ds[t_new]
            it = plan["idx_tiles"][t_new]       # [P, D] new ids
            bt = plan["bias_tiles"][t_new]
            idx1[:, cum[j]:cum[j] + D] = inv_rot[it]   # rotated ids (layer-1)
            idx2[:, cum[j]:cum[j] + D] = it            # global new ids (layer-2)
            biasm[:, cum[j]:cum[j] + D] = bt
        in_maps.append(dict(
            xT=xT_c,
            w_kv1=w_kv1, w_qs1=w_qs1, w2a=w2a, w2b=w2b,
            b_kv1=b_kv1.astype(bf), b_qs1=b_qs1.astype(bf),
            b_kv2=b_kv2.astype(bf), b_qs2=b_qs2.astype(bf),
            idx1_pm=idx1, idx2_pm=idx2,
            bias_pm=biasm.astype(bf),
        ))

    res = run_bass_kernel_spmd(nc, in_maps, core_ids=list(range(N_CORES)))
    kernel.last_results = res

    out_new = np.concatenate([np.asarray(res.results[c]["out"])
                              for c in range(N_CORES)])
    mask = plan["perm"] < N
    out = np.empty((N, D2), np.float32)
    out[plan["perm"][mask]] = out_new[mask]
    return out


# revision 6
# speedup vs baseline: 2.1429x; 1.0058x over previous
"""GraphTransformer 2-layer (TransformerConv x2) on 8 Trainium2 NeuronCores.

Sharding: destination-node partitioning with degree-sorted padded tiles.
  - Pad N=50000 -> N'=50176 (392 tiles of 128 nodes). Sort nodes by
    in-degree, bin-pack the tiles onto 8 cores (49 each, balancing slots).
  - Each core receives x^T (bf16) in a per-core *rotated* node order (its
    own nodes first), computes the full layer-1 K|V table [N',512] bf16
    on-device, plus Q|S for its own nodes (kept in SBUF).
  - Attention per dst-tile: ONE indirect DMA gathers all D neighbors'
    K|V rows ([P, D] offset AP).  Features are stored c-major ([c,h])
    so every big elementwise op keeps a packed 2-byte last dim and runs
    in the DVE 2x mode; reductions are tree-adds (TensorReduce has no
    16-bit fast path).  Padding slots point at row 0 with a -1e30 logit
    bias, so exp() kills them without a max-subtraction pass.
  - Layer-2 K|V [N',20] fp32 is computed from the local h chunk and
    AllGathered (4 MB on the wire).
All indices/degrees/tile shapes are baked in at build time from the actual
inputs. kernel() builds + runs the single-launch SPMD program and
unpermutes the output on the host.
"""

import numpy as np

N_CORES = 8
N = 50000
IN_DIM = 128
D1 = 256            # heads*hid layer1
H1, C1 = 8, 32
D2 = 10             # layer2 out channels (1 head)
P = 128
NEG = -1.0e30


def _plan(edge_index):
    src = np.asarray(edge_index[0], dtype=np.int64)
    dst = np.asarray(edge_index[1], dtype=np.int64)
    deg = np.bincount(dst, minlength=N)
    NP_ = ((N + N_CORES * P - 1) // (N_CORES * P)) * (N_CORES * P)  # 50176
    n_tiles = NP_ // P                                              # 392
    per_core = n_tiles // N_CORES                                   # 49

    degp = np.concatenate([deg, np.zeros(NP_ - N, np.int64)])
    order0 = np.argsort(degp, kind="stable")        # old(padded) ids, deg asc
    tile_of = order0.reshape(n_tiles, P)            # prelim tile -> old ids
    tile_D = degp[tile_of].max(axis=1)

    # bin-pack tiles onto cores: largest-first greedy with capacity
    t_order = np.argsort(-tile_D, kind="stable")
    loads = np.zeros(N_CORES, np.int64)
    counts = np.zeros(N_CORES, np.int64)
    assign = [[] for _ in range(N_CORES)]
    for t in t_order:
        open_cores = [c for c in range(N_CORES) if counts[c] < per_core]
        c = min(open_cores, key=lambda cc: (loads[cc], cc))
        assign[c].append(int(t))
        loads[c] += int(tile_D[t])
        counts[c] += 1
    for c in range(N_CORES):
        assign[c].sort(key=lambda t: int(tile_D[t]))

    final_tiles = [t for c in range(N_CORES) for t in assign[c]]
    perm = tile_of[final_tiles].reshape(-1)         # new id -> old(padded) id
    inv = np.empty(NP_, np.int64)
    inv[perm] = np.arange(NP_)

    Ds = degp[perm].reshape(n_tiles, P).max(axis=1).astype(np.int64)

    # per-(new)tile neighbor tables in NEW ids; pad idx=0, bias=NEG
    dst_new = inv[dst]
    src_new = inv[src]
    eo = np.argsort(dst_new, kind="stable")
    dst_s = dst_new[eo]
    src_s = src_new[eo]
    row_start = np.searchsorted(dst_s, np.arange(NP_))
    row_end = np.searchsorted(dst_s, np.arange(NP_) + 1)

    idx_tiles, bias_tiles = [], []
    for t in range(n_tiles):
        D = int(Ds[t])
        it = np.zeros((P, D), np.int64)
        bt = np.full((P, D), NEG, np.float32)
        for p in range(P):
            s, e = row_start[t * P + p], row_end[t * P + p]
            k = e - s
            it[p, :k] = src_s[s:e]
            bt[p, :k] = 0.0
        idx_tiles.append(it)
        bias_tiles.append(bt)

    return dict(NP=NP_, n_tiles=n_tiles, per_core=per_core, perm=perm,
                inv=inv, Ds=[int(d) for d in Ds], idx_tiles=idx_tiles,
                bias_tiles=bias_tiles)


def _build_program(NP_, per_core, Ds_pos, biases_zero, sim1=False):
    import concourse.bass as bass
    import concourse.mybir as mybir
    from concourse import bacc
    from concourse.tile import TileContext
    from concourse.masks import make_identity

    f32 = mybir.dt.float32
    bf16 = mybir.dt.bfloat16
    i32 = mybir.dt.int32
    NOWN = per_core * P
    Dmax = max(Ds_pos)
    totD = sum(Ds_pos)
    cum = [0]
    for d in Ds_pos:
        cum.append(cum[-1] + d)

    nc = bacc.Bacc("TRN2", target_bir_lowering=False, debug=False,
                   num_devices=1 if sim1 else N_CORES)

    xT = nc.dram_tensor("xT", [IN_DIM, NP_], bf16, kind="ExternalInput")
    w_kv1 = nc.dram_tensor("w_kv1", [IN_DIM, 2 * D1], bf16, kind="ExternalInput")
    w_qs1 = nc.dram_tensor("w_qs1", [IN_DIM, 2 * D1], bf16, kind="ExternalInput")
    # layer-2 weights packed as [kv2_c0 | qs2_c0] ; [kv2_c1 | qs2_c1]
    w2a = nc.dram_tensor("w2a", [P, 2 * (2 * D2)], bf16, kind="ExternalInput")
    w2b = nc.dram_tensor("w2b", [P, 2 * (2 * D2)], bf16, kind="ExternalInput")
    b_kv1 = nc.dram_tensor("b_kv1", [1, 2 * D1], bf16, kind="ExternalInput")
    b_qs1 = nc.dram_tensor("b_qs1", [1, 2 * D1], bf16, kind="ExternalInput")
    b_kv2 = nc.dram_tensor("b_kv2", [1, 2 * D2], bf16, kind="ExternalInput")
    b_qs2 = nc.dram_tensor("b_qs2", [1, 2 * D2], bf16, kind="ExternalInput")
    idx1_pm = nc.dram_tensor("idx1_pm", [P, totD], i32, kind="ExternalInput")
    idx2_pm = nc.dram_tensor("idx2_pm", [P, totD], i32, kind="ExternalInput")
    bias_pm = nc.dram_tensor("bias_pm", [P, totD], bf16, kind="ExternalInput")
    out_d = nc.dram_tensor("out", [NOWN, D2], f32, kind="ExternalOutput")

    kv1_t = nc.dram_tensor("kv1_t", [NP_, 2 * D1], bf16)
    kv2_own = nc.dram_tensor("kv2_own", [NOWN, 2 * D2], f32)
    kv2_full = nc.dram_tensor("kv2_full", [NP_, 2 * D2], f32, addr_space="Shared")

    X = mybir.AxisListType.X
    MUL = mybir.AluOpType.mult
    ADD = mybir.AluOpType.add
    SUB = mybir.AluOpType.subtract
    EXP = mybir.ActivationFunctionType.Exp

    with nc.allow_low_precision(reason="bf16 attention within 2e-2 tolerance"), \
         TileContext(nc) as tc:
        with tc.tile_pool(name="wpool", bufs=1) as wpool:
            w_kv1_s = wpool.tile([IN_DIM, 2 * D1], bf16)
            nc.sync.dma_start(out=w_kv1_s[:], in_=w_kv1[:, :])
            w_qs1_s = wpool.tile([IN_DIM, 2 * D1], bf16)
            nc.sync.dma_start(out=w_qs1_s[:], in_=w_qs1[:, :])
            w2a_s = wpool.tile([P, 2 * (2 * D2)], bf16)
            nc.sync.dma_start(out=w2a_s[:], in_=w2a[:, :])
            w2b_s = wpool.tile([P, 2 * (2 * D2)], bf16)
            nc.sync.dma_start(out=w2b_s[:], in_=w2b[:, :])
            idx1_sb = wpool.tile([P, totD], i32)
            nc.sync.dma_start(out=idx1_sb[:], in_=idx1_pm[:, :])
            idx2_sb = wpool.tile([P, totD], i32)
            nc.sync.dma_start(out=idx2_sb[:], in_=idx2_pm[:, :])
            biasm_sb = wpool.tile([P, totD], bf16)
            nc.sync.dma_start(out=biasm_sb[:], in_=bias_pm[:, :])
            if not biases_zero:
                ones1 = wpool.tile([1, P], bf16)
                nc.vector.memset(ones1[:], 1.0)
                b_kv1_s = wpool.tile([1, 2 * D1], bf16)
                nc.sync.dma_start(out=b_kv1_s[:], in_=b_kv1[:, :])
                b_qs1_s = wpool.tile([1, 2 * D1], bf16)
                nc.sync.dma_start(out=b_qs1_s[:], in_=b_qs1[:, :])
                b_kv2_s = wpool.tile([1, 2 * D2], bf16)
                nc.sync.dma_start(out=b_kv2_s[:], in_=b_kv2[:, :])
                b_qs2_s = wpool.tile([1, 2 * D2], bf16)
                nc.sync.dma_start(out=b_qs2_s[:], in_=b_qs2[:, :])
            ident = wpool.tile([P, P], bf16)
            make_identity(nc, ident[:])
            # SBUF-resident per-core state
            qs1_sb = wpool.tile([P, per_core * 2 * D1], bf16)   # Q|S layer1
            kv2_sb = wpool.tile([P, per_core * 2 * D2], f32)    # K|V layer2
            qs2_sb = wpool.tile([P, per_core * 2 * D2], f32)    # Q|S layer2
            out_sb = wpool.tile([P, per_core * D2], f32)        # final out

            # ================= P1: layer-1 projections =================
            TB = 8                     # tiles per x block
            XB = TB * P                # 1024 columns per load
            with tc.tile_pool(name="p1x", bufs=3) as p1x, \
                 tc.tile_pool(name="p1ps", bufs=8, space="PSUM") as p1ps, \
                 tc.tile_pool(name="p1o", bufs=3) as p1o:
                for blk in range(NP_ // XB):
                    xT_s = p1x.tile([P, XB], bf16, tag="xT")
                    nc.sync.dma_start(out=xT_s[:],
                                      in_=xT[:, blk * XB:(blk + 1) * XB])
                    kvstage = p1o.tile([P, TB * 2 * D1], bf16, tag="kv")
                    for jj in range(TB):
                        t = blk * TB + jj
                        lhsT = xT_s[:, jj * P:(jj + 1) * P]
                        ps = p1ps.tile([P, 2 * D1], f32, tag="ps")
                        nc.tensor.matmul(out=ps[:], lhsT=lhsT, rhs=w_kv1_s[:],
                                         start=True, stop=biases_zero)
                        if not biases_zero:
                            nc.tensor.matmul(out=ps[:], lhsT=ones1[:],
                                             rhs=b_kv1_s[:], start=False, stop=True)
                        dst_ap = kvstage[:, jj * 2 * D1:(jj + 1) * 2 * D1]
                        if t % 2 == 0:
                            nc.scalar.copy(out=dst_ap, in_=ps[:])
                        else:
                            nc.gpsimd.tensor_copy(out=dst_ap, in_=ps[:])
                        if t < per_core:   # own nodes (rotated order)
                            ps2 = p1ps.tile([P, 2 * D1], f32, tag="ps")
                            nc.tensor.matmul(out=ps2[:], lhsT=lhsT, rhs=w_qs1_s[:],
                                             start=True, stop=biases_zero)
                            if not biases_zero:
                                nc.tensor.matmul(out=ps2[:], lhsT=ones1[:],
                                                 rhs=b_qs1_s[:], start=False,
                                                 stop=True)
                            dst2 = qs1_sb[:, t * 2 * D1:(t + 1) * 2 * D1]
                            if t % 2 == 0:
                                nc.gpsimd.tensor_copy(out=dst2, in_=ps2[:])
                            else:
                                nc.scalar.copy(out=dst2, in_=ps2[:])
                    nc.sync.dma_start(
                        out=kv1_t[blk * XB:(blk + 1) * XB, :]
                            .rearrange("(t p) f -> p t f", p=P),
                        in_=kvstage[:].rearrange("p (t f) -> p t f", t=TB))

            # ========== P2: layer-1 attention + layer-2 projections ==========
            with tc.tile_pool(name="kvb", bufs=2) as kvb, \
                 tc.tile_pool(name="prodb", bufs=2) as prodb, \
                 tc.tile_pool(name="small", bufs=3) as small, \
                 tc.tile_pool(name="hps", bufs=2, space="PSUM") as hps, \
                 tc.tile_pool(name="hps2", bufs=2, space="PSUM") as hps2, \
                 tc.tile_pool(name="houtp", bufs=2) as houtp:
                for j in range(per_core):
                    D = Ds_pos[j]
                    c0 = cum[j]
                    # d-range split: DVE handles [0:dv), Pool handles [dv:D)
                    pd = (D * 2) // 10 if D >= 5 else 0
                    dv = D - pd
                    kv_s = kvb.tile([P, Dmax * 2 * D1], bf16, tag="kv")
                    nc.gpsimd.indirect_dma_start(
                        out=kv_s[:, 0:D * 2 * D1],
                        out_offset=None,
                        in_=kv1_t[:, :],
                        in_offset=bass.IndirectOffsetOnAxis(
                            ap=idx1_sb[:, c0:c0 + D], axis=0))
                    kv5 = kv_s[:, 0:D * 2 * D1].rearrange(
                        "p (d x c h) -> p d x c h", d=D, x=2, c=C1)
                    K4 = kv5[:, :, 0, :, :]              # [P, D, 32, 8] bf16
                    V4 = kv5[:, :, 1, :, :]
                    V3 = kv_s[:, 0:D * 2 * D1].rearrange(
                        "p (d x f) -> p d x f", d=D, x=2)[:, :, 1, :]  # [P,D,256]
                    q3 = qs1_sb[:, j * 2 * D1:j * 2 * D1 + D1].rearrange(
                        "p (c h) -> p c h", c=C1)
                    prod = prodb.tile([P, Dmax * D1], bf16, tag="prod")
                    P4 = prod[:, 0:D * D1].rearrange(
                        "p (d c h) -> p d c h", d=D, c=C1)
                    # logits: q . k  (bf16 2x mode: packed last dim = heads)
                    nc.vector.tensor_tensor(
                        out=P4[:, 0:dv], in0=K4[:, 0:dv],
                        in1=q3.unsqueeze(1).to_broadcast([P, dv, C1, H1]),
                        op=MUL)
                    if pd:
                        nc.gpsimd.tensor_tensor(
                            out=P4[:, dv:D], in0=K4[:, dv:D],
                            in1=q3.unsqueeze(1).to_broadcast([P, pd, C1, H1]),
                            op=MUL)
                    cur = C1
                    while cur > 1:
                        nh = cur // 2
                        nc.vector.tensor_tensor(
                            out=P4[:, 0:dv, 0:nh, :], in0=P4[:, 0:dv, 0:nh, :],
                            in1=P4[:, 0:dv, cur - nh:cur, :], op=ADD)
                        if pd:
                            nc.gpsimd.tensor_tensor(
                                out=P4[:, dv:D, 0:nh, :],
                                in0=P4[:, dv:D, 0:nh, :],
                                in1=P4[:, dv:D, cur - nh:cur, :], op=ADD)
                        cur = nh
                    # finalize logits in fp32 with padding bias
                    lg = small.tile([P, Dmax * H1], f32, tag="lg")
                    L3 = lg[:, 0:D * H1].rearrange("p (d h) -> p d h", d=D)
                    nc.vector.tensor_tensor(
                        out=L3, in0=P4[:, :, 0, :],
                        in1=biasm_sb[:, c0:c0 + D].unsqueeze(2)
                            .to_broadcast([P, D, H1]),
                        op=ADD)
                    # softmax (no max-subtraction: logits are O(10) bounded;
                    # pad slots have -1e30 -> exp == 0)
                    e_bf = small.tile([P, Dmax * H1], bf16, tag="ebf")
                    nc.scalar.activation(out=e_bf[:, 0:D * H1],
                                         in_=lg[:, 0:D * H1], func=EXP)
                    sm = small.tile([P, H1], f32, tag="sm")
                    nc.vector.reduce_sum(
                        out=sm[:],
                        in_=e_bf[:, 0:D * H1].rearrange("p (d h) -> p h d", d=D),
                        axis=X)
                    nc.gpsimd.tensor_scalar_add(out=sm[:], in0=sm[:],
                                                scalar1=1e-16)
                    rc = small.tile([P, H1], f32, tag="rc")
                    nc.vector.reciprocal(out=rc[:], in_=sm[:])
                    rcb = small.tile([P, H1], bf16, tag="rcb")
                    nc.gpsimd.tensor_copy(out=rcb[:], in_=rc[:])
                    # weighted V: V *= e  (bf16 2x), then tree-sum over d
                    E3 = e_bf[:, 0:D * H1].rearrange("p (d h) -> p d h", d=D)
                    nc.vector.tensor_tensor(
                        out=V4[:, 0:dv], in0=V4[:, 0:dv],
                        in1=E3[:, 0:dv].unsqueeze(2)
                            .to_broadcast([P, dv, C1, H1]),
                        op=MUL)
                    if pd:
                        nc.gpsimd.tensor_tensor(
                            out=V4[:, dv:D], in0=V4[:, dv:D],
                            in1=E3[:, dv:D].unsqueeze(2)
                                .to_broadcast([P, pd, C1, H1]),
                            op=MUL)
                    cur = D
                    while cur > 1:
                        nh = cur // 2
                        nc.vector.tensor_tensor(
                            out=V3[:, 0:nh, :], in0=V3[:, 0:nh, :],
                            in1=V3[:, cur - nh:cur, :], op=ADD)
                        cur = cur - nh
                    att = houtp.tile([P, D1], f32, tag="att")
                    A3 = att[:].rearrange("p (c h) -> p c h", c=C1)
                    nc.vector.tensor_tensor(
                        out=A3,
                        in0=V3[:, 0, :].rearrange("p (c h) -> p c h", c=C1),
                        in1=rcb[:].unsqueeze(1).to_broadcast([P, C1, H1]),
                        op=MUL)
                    nc.vector.tensor_tensor(
                        out=att[:], in0=att[:],
                        in1=qs1_sb[:, j * 2 * D1 + D1:(j + 1) * 2 * D1],
                        op=ADD)
                    # ELU: h = relu(z) + exp(min(z,0)) - 1
                    zmin = houtp.tile([P, D1], f32, tag="zmin")
                    nc.gpsimd.tensor_scalar_min(out=zmin[:], in0=att[:],
                                                scalar1=0.0)
                    ez = houtp.tile([P, D1], f32, tag="ez")
                    nc.scalar.activation(out=ez[:], in_=zmin[:], func=EXP)
                    nc.vector.tensor_sub(out=att[:], in0=att[:], in1=zmin[:])
                    nc.vector.tensor_add(out=att[:], in0=att[:], in1=ez[:])
                    h_bf = houtp.tile([P, D1], bf16, tag="h")
                    nc.gpsimd.tensor_scalar_add(out=h_bf[:], in0=att[:],
                                                scalar1=-1.0)

                    # ---- layer-2 projections for this tile ----
                    hT0 = hps.tile([P, P], bf16, tag="hT")
                    nc.tensor.transpose(out=hT0[:], in_=h_bf[:, 0:P],
                                        identity=ident[:])
                    hT0s = houtp.tile([P, P], bf16, tag="hT0s")
                    nc.scalar.copy(out=hT0s[:], in_=hT0[:])
                    hT1 = hps.tile([P, P], bf16, tag="hT")
                    nc.tensor.transpose(out=hT1[:], in_=h_bf[:, P:2 * P],
                                        identity=ident[:])
                    hT1s = houtp.tile([P, P], bf16, tag="hT1s")
                    nc.scalar.copy(out=hT1s[:], in_=hT1[:])
                    ps2 = hps2.tile([P, 2 * (2 * D2)], f32, tag="ps2")
                    nc.tensor.matmul(out=ps2[:], lhsT=hT0s[:], rhs=w2a_s[:],
                                     start=True, stop=False)
                    nc.tensor.matmul(out=ps2[:], lhsT=hT1s[:], rhs=w2b_s[:],
                                     start=False, stop=biases_zero)
                    if not biases_zero:
                        nc.tensor.matmul(
                            out=ps2[:, 0:2 * D2], lhsT=ones1[:],
                            rhs=b_kv2_s[:], start=False, stop=False)
                        nc.tensor.matmul(
                            out=ps2[:, 2 * D2:4 * D2], lhsT=ones1[:],
                            rhs=b_qs2_s[:], start=False, stop=True)
                    nc.scalar.copy(out=kv2_sb[:, j * 2 * D2:(j + 1) * 2 * D2],
                                   in_=ps2[:, 0:2 * D2])
                    nc.scalar.copy(out=qs2_sb[:, j * 2 * D2:(j + 1) * 2 * D2],
                                   in_=ps2[:, 2 * D2:4 * D2])
                nc.sync.dma_start(
                    out=kv2_own[:, :].rearrange("(t p) f -> p t f", p=P),
                    in_=kv2_sb[:].rearrange("p (t f) -> p t f", t=per_core))

            # ================= P4: AllGather kv2 =================
            if sim1:
                for c in range(N_CORES):
                    nc.sync.dma_start(
                        out=kv2_full[c * NOWN:(c + 1) * NOWN, :],
                        in_=kv2_own[:, :])
            else:
                nc.gpsimd.collective_compute(
                    "AllGather", mybir.AluOpType.bypass,
                    replica_groups=[list(range(N_CORES))],
                    ins=[kv2_own.ap().opt()],
                    outs=[kv2_full.ap().opt()],
                )

            # ================= P5: layer-2 attention =================
            G = 7                      # tiles per gather group
            n_groups = (per_core + G - 1) // G
            gd_max = max(cum[min(per_core, (g + 1) * G)] - cum[g * G]
                         for g in range(n_groups))
            with tc.tile_pool(name="kvb2", bufs=2) as kvb2, \
                 tc.tile_pool(name="small2", bufs=6) as small2:
                for g in range(n_groups):
                    j0 = g * G
                    j1 = min(per_core, j0 + G)
                    gc0 = cum[j0]
                    gd = cum[j1] - gc0
                    kv2g = kvb2.tile([P, gd_max * 2 * D2], f32, tag="kv2")
                    nc.gpsimd.indirect_dma_start(
                        out=kv2g[:, 0:gd * 2 * D2],
                        out_offset=None,
                        in_=kv2_full[:, :],
                        in_offset=bass.IndirectOffsetOnAxis(
                            ap=idx2_sb[:, gc0:gc0 + gd], axis=0))
                    for j in range(j0, j1):
                        D = Ds_pos[j]
                        c0 = cum[j]
                        off = c0 - gc0
                        G3 = kv2g[:, off * 2 * D2:(off + D) * 2 * D2] \
                            .rearrange("p (d f) -> p d f", d=D)
                        prod2 = small2.tile([P, Dmax * D2], f32, tag="p2")
                        P3 = prod2[:, 0:D * D2].rearrange(
                            "p (d f) -> p d f", d=D)
                        nc.vector.tensor_tensor(
                            out=P3, in0=G3[:, :, 0:D2],
                            in1=qs2_sb[:, j * 2 * D2:j * 2 * D2 + D2]
                                .unsqueeze(1).to_broadcast([P, D, D2]),
                            op=MUL)
                        lg2 = small2.tile([P, Dmax], f32, tag="lg2")
                        nc.vector.reduce_sum(out=lg2[:, 0:D], in_=P3, axis=X)
                        nc.vector.tensor_tensor(
                            out=lg2[:, 0:D], in0=lg2[:, 0:D],
                            in1=biasm_sb[:, c0:c0 + D], op=ADD)
                        e2 = small2.tile([P, Dmax], f32, tag="e2")
                        nc.scalar.activation(out=e2[:, 0:D], in_=lg2[:, 0:D],
                                             func=EXP)
                        sm2 = small2.tile([P, 1], f32, tag="sm2")
                        nc.vector.reduce_sum(out=sm2[:], in_=e2[:, 0:D],
                                             axis=X)
                        nc.gpsimd.tensor_scalar_add(out=sm2[:], in0=sm2[:],
                                                    scalar1=1e-16)
                        rc2 = small2.tile([P, 1], f32, tag="rc2")
                        nc.vector.reciprocal(out=rc2[:], in_=sm2[:])
                        nc.vector.tensor_tensor(
                            out=G3[:, :, D2:2 * D2], in0=G3[:, :, D2:2 * D2],
                            in1=e2[:, 0:D].unsqueeze(2)
                                .to_broadcast([P, D, D2]),
                            op=MUL)
                        att2 = small2.tile([P, D2], f32, tag="att2")
                        nc.vector.reduce_sum(
                            out=att2[:],
                            in_=G3[:, :, D2:2 * D2].transpose([0, 2, 1]),
                            axis=X)
                        nc.vector.tensor_scalar_mul(out=att2[:], in0=att2[:],
                                                    scalar1=rc2[:])
                        nc.vector.tensor_tensor(
                            out=out_sb[:, j * D2:(j + 1) * D2], in0=att2[:],
                            in1=qs2_sb[:, j * 2 * D2 + D2:(j + 1) * 2 * D2],
                            op=ADD)
                nc.sync.dma_start(
                    out=out_d[:, :].rearrange("(t p) f -> p t f", p=P),
                    in_=out_sb[:].rearrange("p (t f) -> p t f", t=per_core))

    nc.compile()
    return nc


_CACHE = {}


def _get_program(NP_, per_core, Ds_pos, biases_zero):
    key = (NP_, per_core, tuple(Ds_pos), biases_zero)
    if key not in _CACHE:
        _CACHE[key] = _build_program(NP_, per_core, Ds_pos, biases_zero)
    return _CACHE[key]


def _cmajor_cols():
    # new column (c*8+h) <- old column (h*32+c)
    return (np.arange(D1).reshape(H1, C1)).T.reshape(-1)


def kernel(**inputs):
    import ml_dtypes
    from concourse.bass_utils import run_bass_kernel_spmd

    bf = ml_dtypes.bfloat16
    x = np.asarray(inputs["x"], np.float32)
    edge_index = np.asarray(inputs["edge_index"])
    plan = _plan(edge_index)
    NP_ = plan["NP"]
    per_core = plan["per_core"]
    Ds = plan["Ds"]
    NOWN = per_core * P

    # position-aligned degrees (SPMD: one program for all cores)
    Ds_pos = [max(Ds[c * per_core + j] for c in range(N_CORES))
              for j in range(per_core)]
    totD = sum(Ds_pos)
    cum = np.zeros(per_core + 1, np.int64)
    cum[1:] = np.cumsum(Ds_pos)

    cm = _cmajor_cols()
    s1 = 1.0 / np.sqrt(np.float32(C1))
    s2 = 1.0 / np.sqrt(np.float32(D2))
    w1k = np.asarray(inputs["w1k"], np.float32)[:, cm]
    w1v = np.asarray(inputs["w1v"], np.float32)[:, cm]
    w1q = np.asarray(inputs["w1q"], np.float32)[:, cm] * s1
    w1s = np.asarray(inputs["w1s"], np.float32)[:, cm]
    w_kv1 = np.ascontiguousarray(
        np.concatenate([w1k, w1v], axis=1)).astype(bf)
    w_qs1 = np.ascontiguousarray(
        np.concatenate([w1q, w1s], axis=1)).astype(bf)
    # layer-2 weights: rows permuted to c-major (h is c-major), packed as
    # [kv2 | qs2] per 128-row chunk
    w2k = np.asarray(inputs["w2k"], np.float32)[cm, :] * 1.0
    w2v = np.asarray(inputs["w2v"], np.float32)[cm, :]
    w2q = np.asarray(inputs["w2q"], np.float32)[cm, :] * s2
    w2s = np.asarray(inputs["w2s"], np.float32)[cm, :]
    wkv2 = np.concatenate([w2k, w2v], axis=1)      # [256, 20]
    wqs2 = np.concatenate([w2q, w2s], axis=1)      # [256, 20]
    w2a = np.ascontiguousarray(
        np.concatenate([wkv2[0:P], wqs2[0:P]], axis=1)).astype(bf)
    w2b = np.ascontiguousarray(
        np.concatenate([wkv2[P:2 * P], wqs2[P:2 * P]], axis=1)).astype(bf)
    b_kv1 = np.concatenate([np.asarray(inputs["b1k"], np.float32)[cm],
                            np.asarray(inputs["b1v"], np.float32)[cm]])[None]
    b_qs1 = np.concatenate([np.asarray(inputs["b1q"], np.float32)[cm] * s1,
                            np.asarray(inputs["b1s"], np.float32)[cm]])[None]
    b_kv2 = np.concatenate([np.asarray(inputs["b2k"], np.float32),
                            np.asarray(inputs["b2v"], np.float32)])[None]
    b_qs2 = np.concatenate([np.asarray(inputs["b2q"], np.float32) * s2,
                            np.asarray(inputs["b2s"], np.float32)])[None]
    biases_zero = all(not np.any(b) for b in (b_kv1, b_qs1, b_kv2, b_qs2))

    nc = _get_program(NP_, per_core, Ds_pos, biases_zero)

    xpad = np.concatenate([x, np.zeros((NP_ - N, IN_DIM), np.float32)])
    x_new = xpad[plan["perm"]]
    xT_new = np.ascontiguousarray(x_new.T).astype(bf)

    in_maps = []
    for c in range(N_CORES):
        own0 = c * NOWN
        rot = np.concatenate([np.arange(own0, own0 + NOWN),
                              np.arange(0, own0),
                              np.arange(own0 + NOWN, NP_)])
        inv_rot = np.empty(NP_, np.int64)
        inv_rot[rot] = np.arange(NP_)
        xT_c = np.ascontiguousarray(xT_new[:, rot])
        idx1 = np.zeros((P, totD), np.int32)
        idx2 = np.zeros((P, totD), np.int32)
        biasm = np.full((P, totD), NEG, np.float32)
        for j in range(per_core):
            t_new = c * per_core + j
            D = Ds[t_new]
            it = plan["idx_tiles"][t_new]       # [P, D] new ids
            bt = plan["bias_tiles"][t_new]
            idx1[:, cum[j]:cum[j] + D] = inv_rot[it]   # rotated ids (layer-1)
            idx2[:, cum[j]:cum[j] + D] = it            # global new ids (layer-2)
            biasm[:, cum[j]:cum[j] + D] = bt
        in_maps.append(dict(
            xT=xT_c,
            w_kv1=w_kv1, w_qs1=w_qs1, w2a=w2a, w2b=w2b,
            b_kv1=b_kv1.astype(bf), b_qs1=b_qs1.astype(bf),
            b_kv2=b_kv2.astype(bf), b_qs2=b_qs2.astype(bf),
            idx1_pm=idx1, idx2_pm=idx2,
            bias_pm=biasm.astype(bf),
        ))

    res = run_bass_kernel_spmd(nc, in_maps, core_ids=list(range(N_CORES)))
    kernel.last_results = res

    out_new = np.concatenate([np.asarray(res.results[c]["out"])
                              for c in range(N_CORES)])
    mask = plan["perm"] < N
    out = np.empty((N, D2), np.float32)
    out[plan["perm"][mask]] = out_new[mask]
    return out
